# revision 18
# baseline (speedup 1.0000x reference)
"""Trainium2 Bass kernel for the additive-attention problem (V4).

reference math:
    rec[b,h]    = sum_r rnn_state[b,r] * W_rec[h,r]
    scores[t,b] = sum_h tanh(enc[t,b,h] + rec[b,h]) * w_score[h] + b_score + mask[t,b]
    out         = softmax(scores, axis=t)          # (T, B) float32

Sharding: data-parallel over B across 8 cores (BL=4 batch columns per core).
Softmax is over T (core-local) -> no collectives.

V4 structure (baseline V3.8 was ACT-bound at 58.3us busy + 15.8us startup
+ 8.6us tail = 82.4us):
  - tanh offload: tiles (b,hc=2) for b in 0..2 are computed on the DVE
    with a 9-op fp16 chain  y = t*g(|t|),  t = clamp(x+rec, +-C),
    g cubic in |t| (end-to-end rel err 4.3e-3 measured vs 2e-2 gate).
    ACT keeps 13 of 16 tiles -> ~47us; DVE chain 3x13.3us -> ~42us.
  - per-b PSUM accumulation: the 4 hc partial score sets accumulate in
    PSUM (start=False everywhere, mask pre-written into PSUM by DVE),
    killing the 5-op DVE combine chain per b.
  - per-b incremental tail: exp_b (ACT, PSUM src) -> transpose_b (PE,
    partition block b*32) -> rowsum_b (DVE, bf16) during the main loop;
    only b=3's chain + mblk-matmul/recip/scale/DMA remain at the end.
  - startup: first tiles sliced (1024/1024/2048), DMAs spread across 5
    rings (vector: rec-chain weights; sync: ACT enc stream; scalar:
    mask + b0hc1; tensor: the 3 DVE tiles; gpsimd: mblk).
  - emission order per engine is tuned so no engine head-of-line-blocks
    a critical dependency (exps are placed late in the ACT queue, the
    transposes late in the PE queue).
"""

import numpy as np

T, B, H, R = 4096, 32, 512, 512
NCORES = 8
BL = B // NCORES          # 4 local batch columns
HC = H // 128             # 4 h-chunks
RC = R // 128             # 4 r-chunks
NTC = T // 128            # 32 t-chunks of 128

# DVE tanh approx: tanh(x) ~= t*(C0 + C1*s + C2*s^2 + C3*s^3),
# t = clamp(x, +-CLAMP), s = |t|  (minimax fit on [0, CLAMP])
CLAMP = 2.6
G0, G1, G2, G3 = 1.07877621, -0.30459219, -0.03455844, 0.01874759
# which (b, hc) tiles run on the DVE instead of ACT
OFFLOAD = ((0, 2), (1, 2), (2, 2))

_GRAPH = None


def _build_graph():
    import concourse.bass as bass
    import concourse.tile as tile
    from concourse import bacc, mybir
    from concourse.masks import make_identity

    f32 = mybir.dt.float32
    f16 = mybir.dt.float16
    bf16 = mybir.dt.bfloat16
    AF = mybir.ActivationFunctionType
    OP = mybir.AluOpType
    nc = bacc.Bacc()

    encT = nc.declare_dram_parameter("encT", [BL, H, T], f16, isOutput=False)
    maskd = nc.declare_dram_parameter("maskd", [128, BL, NTC], f32, isOutput=False)
    rnnd = nc.declare_dram_parameter("rnnd", [128, RC, BL], f16, isOutput=False)
    wrecd = nc.declare_dram_parameter(
        "wrecd", [HC, 128, RC, 128], f16, isOutput=False
    )
    wcold = nc.declare_dram_parameter("wcold", [128, HC], f16, isOutput=False)
    mblkd = nc.declare_dram_parameter("mblk", [128, 128], bf16, isOutput=False)
    out = nc.declare_dram_parameter("out", [BL, T], f32, isOutput=True)

    offl = set(OFFLOAD)

    with tile.TileContext(nc) as tc:
        with (
            tc.tile_pool(name="singles", bufs=1) as singles,
            tc.tile_pool(name="xpool", bufs=6) as xpool,
            tc.tile_pool(name="xdpool", bufs=3) as xdpool,
            tc.tile_pool(name="ypool", bufs=6) as ypool,
            tc.tile_pool(name="qpool", bufs=1) as qpool,
            tc.tile_pool(name="spool", bufs=1, space="PSUM") as spool,
            tc.tile_pool(name="pppool", bufs=1, space="PSUM") as pppool,
            tc.tile_pool(name="spool1", bufs=1, space="PSUM") as spool1,
        ):
            ident = singles.tile([128, 128], f32)
            make_identity(nc, ident[:])
            # dummy tanh: forces ACT_TABLE_LOAD at t~=0 (no DMA dependency)
            dummy = singles.tile([128, 1], f32)
            nc.scalar.activation(
                out=dummy[:], in_=ident[:, 0:1], func=AF.Tanh,
            )

            encv = encT.rearrange("b (hc p) t -> b hc p t", p=128)

            # ---------------- DMA issue plan ----------------
            # Only 3 rings exist: sync (SP), scalar (Activation), gpsimd
            # (SWDGE).  scalar-ring issues cost ~0.7us of ACT-engine time
            # each, so they are confined to the idle startup window.

            # sync ring: rec-chain inputs first (tiny, needed by the first
            # ACTIVATE's bias), then the ACT enc stream.  First tile in 3
            # slices so the first ACTIVATE starts as early as possible.
            wrec_sb = singles.tile([128, HC, RC, 128], f16)
            rnn_sb = singles.tile([128, RC, BL], f16)
            wcol = singles.tile([128, HC], f16)
            nc.sync.dma_start(out=wrec_sb[:, 0], in_=wrecd[0])
            nc.sync.dma_start(out=rnn_sb[:], in_=rnnd[:])
            nc.sync.dma_start(out=wcol[:], in_=wcold[:])
            X00 = xpool.tile([128, T], f16, name="X")
            nc.sync.dma_start(out=X00[:, 0:1024], in_=encv[0, 0, :, 0:1024])
            nc.sync.dma_start(out=X00[:, 1024:2048], in_=encv[0, 0, :, 1024:2048])
            nc.sync.dma_start(out=X00[:, 2048:T], in_=encv[0, 0, :, 2048:T])
            nc.sync.dma_start(out=wrec_sb[:, 2], in_=wrecd[2])
            nc.sync.dma_start(out=wrec_sb[:, 1], in_=wrecd[1])
            nc.sync.dma_start(out=wrec_sb[:, 3], in_=wrecd[3])

            # scalar ring (issues ride the idle ACT startup window):
            # b0hc1 halves + the first DVE tile
            X01 = xpool.tile([128, T], f16, name="X")
            nc.scalar.dma_start(out=X01[:, :2048], in_=encv[0, 1, :, :2048])
            nc.scalar.dma_start(out=X01[:, 2048:], in_=encv[0, 1, :, 2048:])
            Xdve = {}
            Xdve[(0, 2)] = xdpool.tile([128, T], f16, name="XD")
            nc.scalar.dma_start(out=Xdve[(0, 2)][:], in_=encv[0, 2, :, :])

            # gpsimd ring: mask (needed by ~11us for the PSUM init), the
            # two later DVE tiles (loose deadlines), and mblk (end only)
            mask_sb = singles.tile([128, BL, NTC], f32)
            nc.gpsimd.dma_start(out=mask_sb[:], in_=maskd[:])
            Xdve[(1, 2)] = xdpool.tile([128, T], f16, name="XD")
            nc.gpsimd.dma_start(out=Xdve[(1, 2)][:], in_=encv[1, 2, :, :])
            Xdve[(2, 2)] = xdpool.tile([128, T], f16, name="XD")
            nc.gpsimd.dma_start(out=Xdve[(2, 2)][:], in_=encv[2, 2, :, :])
            mblk = singles.tile([128, 128], bf16)
            nc.gpsimd.dma_start(out=mblk[:], in_=mblkd[:])

            # ---------------- rec chain (PE + DVE copies) ----------------
            # rec.T[h, b] = sum_r W_rec[h, r] * rnn[b, r], per h-chunk
            rec_sb = singles.tile([128, HC, BL], f32)

            def rec_chunk(hc):
                rp = spool.tile([128, BL], f32, tag="rec")
                for rc in range(RC):
                    nc.tensor.matmul(
                        rp[:],
                        lhsT=wrec_sb[:, hc, rc],
                        rhs=rnn_sb[:, rc, :],
                        start=(rc == 0),
                        stop=(rc == RC - 1),
                    )
                nc.vector.tensor_copy(out=rec_sb[:, hc, :], in_=rp[:])

            rec_chunk(0)
            rec_chunk(2)
            rec_chunk(1)
            rec_chunk(3)

            # ---------------- per-b PSUM tiles + mask init ----------------
            Pp = [pppool.tile([128, NTC], f32, tag=f"pp{b}", name=f"pp{b}") for b in range(BL)]
            for b in range(BL):
                nc.vector.tensor_copy(out=Pp[b][:], in_=mask_sb[:, b, :])

            # ---------------- main loop ----------------
            E = singles.tile([128, BL, NTC], f32)       # exp outputs per b
            attT = spool1.tile([128, 128], f32, tag="attT")
            R128 = singles.tile([128, 1], bf16)         # per-(b,tc) sums

            def score_cols(b, hc, Y, cols, stop):
                """PE reduction over one h-chunk for t-chunks in `cols`."""
                for tcng in cols:
                    nc.tensor.matmul(
                        Pp[b][:, tcng : tcng + 1],
                        lhsT=Y[:, tcng * 128 : (tcng + 1) * 128],
                        rhs=wcol[:, hc : hc + 1],
                        start=False,
                        stop=stop,
                        skip_group_check=True,
                    )

            def act_tile(b, hc, X, slices):
                Y = ypool.tile([128, T], f16, name="Y")
                for s0, s1 in slices:
                    nc.scalar.activation(
                        out=Y[:, s0:s1],
                        in_=X[:, s0:s1],
                        func=AF.Tanh,
                        bias=rec_sb[:, hc, b : b + 1],
                    )
                return Y

            def dve_tile(b, hc, X):
                """y = t*g(|t|), t = clamp(x + rec, +-C) on the DVE.
                7 ops: 4 tensor_scalar (4x rate) + 3 scalar_tensor_tensor
                (2x rate):  g = ((G3*s + G2)*s + G1)*s + G0 via the
                (w + c)*s ladder, then y = (w + G0)*t."""
                rb = rec_sb[:, hc, b : b + 1]
                t0 = qpool.tile([128, T], f16)
                nc.vector.tensor_scalar(
                    out=t0[:], in0=X[:], scalar1=rb, scalar2=CLAMP,
                    op0=OP.add, op1=OP.min,
                )
                t = qpool.tile([128, T], f16)
                nc.vector.tensor_scalar(
                    out=t[:], in0=t0[:], scalar1=-CLAMP, scalar2=None,
                    op0=OP.max,
                )
                s = qpool.tile([128, T], f16)
                # |t| by clearing the fp16 sign bit (abs_max fails the
                # HW ISA check for tensor_scalar)
                nc.vector.tensor_scalar(
                    out=s[:].bitcast(mybir.dt.uint16),
                    in0=t[:].bitcast(mybir.dt.uint16),
                    scalar1=0x7FFF, scalar2=None,
                    op0=OP.bitwise_and,
                )
                w = qpool.tile([128, T], f16)
                nc.vector.tensor_scalar(
                    out=w[:], in0=s[:], scalar1=G3, scalar2=G2,
                    op0=OP.mult, op1=OP.add,
                )
                w2 = qpool.tile([128, T], f16)
                nc.vector.scalar_tensor_tensor(
                    out=w2[:], in0=w[:], scalar=0.0, in1=s[:],
                    op0=OP.add, op1=OP.mult,
                )
                nc.vector.scalar_tensor_tensor(
                    out=w[:], in0=w2[:], scalar=G1, in1=s[:],
                    op0=OP.add, op1=OP.mult,
                )
                Y = ypool.tile([128, T], f16, name="Y")
                nc.vector.scalar_tensor_tensor(
                    out=Y[:], in0=w[:], scalar=G0, in1=t[:],
                    op0=OP.add, op1=OP.mult,
                )
                return Y

            def exp_b(b):
                nc.scalar.activation(
                    out=E[:, b, :], in_=Pp[b][:], func=AF.Exp,
                )

            def transpose_all():
                # E free layout is (b, tc) -> one transpose gives the
                # (b,tc)-partition layout directly (PE transpose output
                # base partition must be 0/32/64, so per-b is impossible)
                nc.tensor.transpose(
                    out=attT[:],
                    in_=E[:].rearrange("p b tc -> p (b tc)"),
                    identity=ident[:],
                )

            def rowsum_all():
                with nc.allow_low_precision(reason="bf16 rowsums, f32 accum"):
                    nc.vector.tensor_reduce(
                        out=R128[:],
                        in_=attT[:],
                        axis=mybir.AxisListType.X,
                        op=OP.add,
                    )

            allc = list(range(NTC))
            Ys = {}
            slices_for = {
                (0, 0): [(0, 1024), (1024, 2048), (2048, T)],
                (0, 1): [(0, 2048), (2048, T)],
            }

            def emit_act(b, hc):
                if (b, hc) == (0, 0):
                    X = X00
                elif (b, hc) == (0, 1):
                    X = X01
                else:
                    X = xpool.tile([128, T], f16, name="X")
                    nc.sync.dma_start(out=X[:], in_=encv[b, hc, :, :])
                Ys[(b, hc)] = act_tile(
                    b, hc, X, slices_for.get((b, hc), [(0, T)])
                )

            # Global emission order IS the dependency DAG (the framework
            # derives deps from data flow at emission time); each engine
            # executes its own subsequence in FIFO order.  This order is
            # arranged so no engine head-of-line-blocks a critical dep.
            emit_act(0, 0)
            emit_act(0, 1)
            score_cols(0, 0, Ys[(0, 0)], allc, False)
            score_cols(0, 1, Ys[(0, 1)], allc, False)
            emit_act(0, 3)
            score_cols(0, 3, Ys[(0, 3)], allc, False)
            Ys[(0, 2)] = dve_tile(0, 2, Xdve[(0, 2)])
            score_cols(0, 2, Ys[(0, 2)], allc, True)     # closes Pp[0]
            emit_act(1, 0)
            emit_act(1, 1)
            score_cols(1, 0, Ys[(1, 0)], allc, False)
            score_cols(1, 1, Ys[(1, 1)], allc, False)
            emit_act(1, 3)
            score_cols(1, 3, Ys[(1, 3)], allc, False)
            Ys[(1, 2)] = dve_tile(1, 2, Xdve[(1, 2)])
            score_cols(1, 2, Ys[(1, 2)], allc, True)     # closes Pp[1]
            exp_b(0)
            emit_act(2, 0)
            emit_act(2, 1)
            score_cols(2, 0, Ys[(2, 0)], allc, False)
            score_cols(2, 1, Ys[(2, 1)], allc, False)
            emit_act(2, 3)
            score_cols(2, 3, Ys[(2, 3)], allc, False)
            Ys[(2, 2)] = dve_tile(2, 2, Xdve[(2, 2)])
            exp_b(1)
            emit_act(3, 0)
            emit_act(3, 1)
            score_cols(3, 0, Ys[(3, 0)], allc, False)
            score_cols(3, 1, Ys[(3, 1)], allc, False)
            score_cols(2, 2, Ys[(2, 2)], allc, True)     # closes Pp[2]
            emit_act(3, 2)
            score_cols(3, 2, Ys[(3, 2)], allc, False)
            # last tile in halves so the post-tanh PE tail is short;
            # exp_b2 rides between the halves
            X33 = xpool.tile([128, T], f16, name="X")
            nc.sync.dma_start(out=X33[:], in_=encv[3, 3, :, :])
            Y33 = ypool.tile([128, T], f16, name="Y")
            nc.scalar.activation(
                out=Y33[:, 0:2048], in_=X33[:, 0:2048], func=AF.Tanh,
                bias=rec_sb[:, 3, 3:4],
            )
            score_cols(3, 3, Y33, list(range(16)), False)
            exp_b(2)
            nc.scalar.activation(
                out=Y33[:, 2048:T], in_=X33[:, 2048:T], func=AF.Tanh,
                bias=rec_sb[:, 3, 3:4],
            )
            score_cols(3, 3, Y33, list(range(16, NTC)), True)  # closes Pp[3]
            exp_b(3)
            transpose_all()
            rowsum_all()

            # DVE poly ops were emitted on the vector queue above; the
            # rowsums sit after them and before the final chain.

            # ---------------- final normalization ----------------
            denom = spool.tile([128, 1], f32, tag="denom")
            nc.tensor.matmul(
                denom[:], lhsT=mblk[:], rhs=R128[:], start=True, stop=True
            )
            recip = singles.tile([128, 1], f32)
            nc.vector.reciprocal(out=recip[:], in_=denom[:])
            att_out = singles.tile([128, 128], f32)
            nc.vector.tensor_scalar_mul(
                out=att_out[:], in0=attT[:], scalar1=recip[:]
            )
            # partition p = (b, tc) holds 128 contiguous t values for col b
            nc.sync.dma_start(
                out=out.rearrange("b (tc tp) -> (b tc) tp", tp=128),
                in_=att_out[:],
            )

    nc.compile()
    return nc


def _get_graph():
    global _GRAPH
    if _GRAPH is None:
        _GRAPH = _build_graph()
    return _GRAPH


def make_in_maps(enc, mask, rnn_state, W_rec, w_score):
    import ml_dtypes

    enc16 = np.asarray(enc, dtype=np.float16)
    # [T, B, H] -> [B, H, T]
    encT_full = np.ascontiguousarray(enc16.transpose(1, 2, 0))
    # [HC, 128, RC, 128]: wrecd[hc, p, rc, w] = W_rec[hc*128+w, rc*128+p]
    wrecd = np.ascontiguousarray(
        W_rec.T.astype(np.float16)
        .reshape(RC, 128, HC, 128)
        .transpose(2, 1, 0, 3)
    )
    wcol = np.ascontiguousarray(
        np.asarray(w_score, dtype=np.float16).reshape(HC, 128).T
    )
    # block-diag ones: mblk[p, q] = (p//32 == q//32)
    mblk = (np.arange(128)[:, None] // 32 == np.arange(128)[None, :] // 32)
    mblk = np.ascontiguousarray(mblk.astype(ml_dtypes.bfloat16))
    in_maps = []
    for c in range(NCORES):
        sl = slice(c * BL, (c + 1) * BL)
        # rnnd[p, rc, b] = rnn_state[c*BL+b, rc*128+p]
        rnnd = np.ascontiguousarray(
            rnn_state[sl].astype(np.float16).T.reshape(RC, 128, BL)
            .transpose(1, 0, 2)
        )
        # maskd[p, b, tc] = mask[tc*128+p, c*BL+b]
        maskc = np.ascontiguousarray(
            np.asarray(mask[:, sl], dtype=np.float32).reshape(NTC, 128, BL)
            .transpose(1, 2, 0)
        )
        in_maps.append(
            {
                "encT": np.ascontiguousarray(encT_full[sl]),
                "maskd": maskc,
                "rnnd": rnnd,
                "wrecd": wrecd,
                "wcold": wcol,
                "mblk": mblk,
            }
        )
    return in_maps


def kernel(
    encoded_contribution,
    mask,
    rnn_state,
    prev_att_weights,
    W_rec,
    w_score,
    b_score,
):
    from concourse.bass_utils import run_bass_kernel_spmd

    nc = _get_graph()
    in_maps = make_in_maps(
        np.asarray(encoded_contribution),
        np.asarray(mask),
        np.asarray(rnn_state),
        np.asarray(W_rec),
        np.asarray(w_score),
    )
    res = run_bass_kernel_spmd(nc, in_maps, list(range(NCORES)))
    outs = [np.asarray(res.results[c]["out"]) for c in range(NCORES)]
    return np.concatenate([o.T for o in outs], axis=1).astype(np.float32)


# revision 20
# speedup vs baseline: 1.0638x; 1.0638x over previous
"""Trainium2 Bass kernel for the additive-attention problem (V4).

reference math:
    rec[b,h]    = sum_r rnn_state[b,r] * W_rec[h,r]
    scores[t,b] = sum_h tanh(enc[t,b,h] + rec[b,h]) * w_score[h] + b_score + mask[t,b]
    out         = softmax(scores, axis=t)          # (T, B) float32

Sharding: data-parallel over B across 8 cores (BL=4 batch columns per core).
Softmax is over T (core-local) -> no collectives.

V4 structure (baseline V3.8 was ACT-bound at 58.3us busy + 15.8us startup
+ 8.6us tail = 82.4us):
  - tanh offload: tiles (b,hc=2) for b in 0..2 are computed on the DVE
    with a 9-op fp16 chain  y = t*g(|t|),  t = clamp(x+rec, +-C),
    g cubic in |t| (end-to-end rel err 4.3e-3 measured vs 2e-2 gate).
    ACT keeps 13 of 16 tiles -> ~47us; DVE chain 3x13.3us -> ~42us.
  - per-b PSUM accumulation: the 4 hc partial score sets accumulate in
    PSUM (start=False everywhere, mask pre-written into PSUM by DVE),
    killing the 5-op DVE combine chain per b.
  - per-b incremental tail: exp_b (ACT, PSUM src) -> transpose_b (PE,
    partition block b*32) -> rowsum_b (DVE, bf16) during the main loop;
    only b=3's chain + mblk-matmul/recip/scale/DMA remain at the end.
  - startup: first tiles sliced (1024/1024/2048), DMAs spread across 5
    rings (vector: rec-chain weights; sync: ACT enc stream; scalar:
    mask + b0hc1; tensor: the 3 DVE tiles; gpsimd: mblk).
  - emission order per engine is tuned so no engine head-of-line-blocks
    a critical dependency (exps are placed late in the ACT queue, the
    transposes late in the PE queue).
"""

import numpy as np

T, B, H, R = 4096, 32, 512, 512
NCORES = 8
BL = B // NCORES          # 4 local batch columns
HC = H // 128             # 4 h-chunks
RC = R // 128             # 4 r-chunks
NTC = T // 128            # 32 t-chunks of 128

# DVE tanh approx: tanh(x) ~= t*(C0 + C1*s + C2*s^2 + C3*s^3),
# t = clamp(x, +-CLAMP), s = |t|  (minimax fit on [0, CLAMP])
CLAMP = 2.6
G0, G1, G2, G3 = 1.07877621, -0.30459219, -0.03455844, 0.01874759
# which (b, hc) tiles run on the DVE instead of ACT
OFFLOAD = ((0, 2), (1, 2), (2, 2))

_GRAPH = None


def _build_graph():
    import concourse.bass as bass
    import concourse.tile as tile
    from concourse import bacc, mybir
    from concourse.masks import make_identity

    f32 = mybir.dt.float32
    f16 = mybir.dt.float16
    bf16 = mybir.dt.bfloat16
    AF = mybir.ActivationFunctionType
    OP = mybir.AluOpType
    nc = bacc.Bacc()

    encT = nc.declare_dram_parameter("encT", [BL, H, T], f16, isOutput=False)
    maskd = nc.declare_dram_parameter("maskd", [128, BL, NTC], f32, isOutput=False)
    rnnd = nc.declare_dram_parameter("rnnd", [128, RC, BL], f16, isOutput=False)
    wrecd = nc.declare_dram_parameter(
        "wrecd", [HC, 128, RC, 128], f16, isOutput=False
    )
    wcold = nc.declare_dram_parameter("wcold", [128, HC], f16, isOutput=False)
    mblkd = nc.declare_dram_parameter("mblk", [128, 128], bf16, isOutput=False)
    out = nc.declare_dram_parameter("out", [BL, T], f32, isOutput=True)

    offl = set(OFFLOAD)

    with tile.TileContext(nc) as tc:
        with (
            tc.tile_pool(name="singles", bufs=1) as singles,
            tc.tile_pool(name="xpool", bufs=6) as xpool,
            tc.tile_pool(name="xdpool", bufs=3) as xdpool,
            tc.tile_pool(name="ypool", bufs=6) as ypool,
            tc.tile_pool(name="qpool", bufs=1) as qpool,
            tc.tile_pool(name="spool", bufs=1, space="PSUM") as spool,
            tc.tile_pool(name="pppool", bufs=1, space="PSUM") as pppool,
            tc.tile_pool(name="spool1", bufs=1, space="PSUM") as spool1,
        ):
            ident = singles.tile([128, 128], f32)
            make_identity(nc, ident[:])
            # dummy tanh: forces ACT_TABLE_LOAD at t~=0 (no DMA dependency)
            dummy = singles.tile([128, 1], f32)
            nc.scalar.activation(
                out=dummy[:], in_=ident[:, 0:1], func=AF.Tanh,
            )

            encv = encT.rearrange("b (hc p) t -> b hc p t", p=128)

            # ---------------- DMA issue plan ----------------
            # Only 3 rings exist: sync (SP), scalar (Activation), gpsimd
            # (SWDGE).  The DMA/HBM pipe runs at only ~30-50 GB/s
            # aggregate for the first ~15us, then steps to full rate --
            # so the early queue must contain ONLY what gates the first
            # ACTIVATE (rec-chain smalls + X00).  Everything else is
            # issued later, riding the fast window.
            wrec_sb = singles.tile([128, HC, RC, 128], f16)
            rnn_sb = singles.tile([128, RC, BL], f16)
            wcol = singles.tile([128, HC], f16)
            nc.sync.dma_start(out=rnn_sb[:], in_=rnnd[:])
            nc.sync.dma_start(out=wrec_sb[:, 0], in_=wrecd[0])
            X00 = xpool.tile([128, T], f16, name="X")
            nc.sync.dma_start(out=X00[:, 0:1024], in_=encv[0, 0, :, 0:1024])
            nc.sync.dma_start(out=X00[:, 1024:2048], in_=encv[0, 0, :, 1024:2048])
            nc.sync.dma_start(out=X00[:, 2048:T], in_=encv[0, 0, :, 2048:T])
            nc.sync.dma_start(out=wcol[:], in_=wcold[:])
            nc.sync.dma_start(out=wrec_sb[:, 2], in_=wrecd[2])
            nc.sync.dma_start(out=wrec_sb[:, 1], in_=wrecd[1])
            nc.sync.dma_start(out=wrec_sb[:, 3], in_=wrecd[3])

            # gpsimd ring: mask only (64KB, needed by ~13us for the PSUM
            # init).  The DVE tiles X12/X22 + mblk are issued mid-loop.
            mask_sb = singles.tile([128, BL, NTC], f32)
            nc.gpsimd.dma_start(out=mask_sb[:], in_=maskd[:])
            Xdve = {}
            Xdve[(0, 2)] = xdpool.tile([128, T], f16, name="XD")
            Xdve[(1, 2)] = xdpool.tile([128, T], f16, name="XD")
            Xdve[(2, 2)] = xdpool.tile([128, T], f16, name="XD")
            mblk = singles.tile([128, 128], bf16)

            # ---------------- rec chain (PE + DVE copies) ----------------
            # rec.T[h, b] = sum_r W_rec[h, r] * rnn[b, r], per h-chunk
            rec_sb = singles.tile([128, HC, BL], f32)

            def rec_chunk(hc):
                rp = spool.tile([128, BL], f32, tag="rec")
                for rc in range(RC):
                    nc.tensor.matmul(
                        rp[:],
                        lhsT=wrec_sb[:, hc, rc],
                        rhs=rnn_sb[:, rc, :],
                        start=(rc == 0),
                        stop=(rc == RC - 1),
                    )
                nc.vector.tensor_copy(out=rec_sb[:, hc, :], in_=rp[:])

            rec_chunk(0)
            rec_chunk(2)
            rec_chunk(1)
            rec_chunk(3)

            # ---------------- per-b PSUM tiles + mask init ----------------
            Pp = [pppool.tile([128, NTC], f32, tag=f"pp{b}", name=f"pp{b}") for b in range(BL)]
            for b in range(BL):
                nc.vector.tensor_copy(out=Pp[b][:], in_=mask_sb[:, b, :])

            # ---------------- main loop ----------------
            E = singles.tile([128, BL, NTC], f32)       # exp outputs per b
            attT = spool1.tile([128, 128], f32, tag="attT")
            R128 = singles.tile([128, 1], bf16)         # per-(b,tc) sums

            def score_cols(b, hc, Y, cols, stop):
                """PE reduction over one h-chunk for t-chunks in `cols`."""
                for tcng in cols:
                    nc.tensor.matmul(
                        Pp[b][:, tcng : tcng + 1],
                        lhsT=Y[:, tcng * 128 : (tcng + 1) * 128],
                        rhs=wcol[:, hc : hc + 1],
                        start=False,
                        stop=stop,
                        skip_group_check=True,
                    )

            def act_tile(b, hc, X, slices):
                Y = ypool.tile([128, T], f16, name="Y")
                for s0, s1 in slices:
                    nc.scalar.activation(
                        out=Y[:, s0:s1],
                        in_=X[:, s0:s1],
                        func=AF.Tanh,
                        bias=rec_sb[:, hc, b : b + 1],
                    )
                return Y

            def dve_tile(b, hc, X, Y, lo, hi):
                """Y[:, lo:hi] = t*g(|t|), t = clamp(x + rec, +-C), on the
                DVE.  9 ops: 6 tensor_scalar (4x rate) + 3 tensor_tensor
                (2x):  g = ((G3*s + G2)*s + G1)*s + G0, then y = g*t.
                (scalar_tensor_tensor only has a 1x uop - measured.)"""
                rb = rec_sb[:, hc, b : b + 1]
                sl = slice(lo, hi)
                t0 = qpool.tile([128, T], f16)
                nc.vector.tensor_scalar(
                    out=t0[:, sl], in0=X[:, sl], scalar1=rb, scalar2=CLAMP,
                    op0=OP.add, op1=OP.min,
                )
                t = qpool.tile([128, T], f16)
                nc.vector.tensor_scalar(
                    out=t[:, sl], in0=t0[:, sl], scalar1=-CLAMP, scalar2=None,
                    op0=OP.max,
                )
                s = qpool.tile([128, T], f16)
                # |t| by clearing the fp16 sign bit (abs_max fails the
                # HW ISA check for tensor_scalar)
                nc.vector.tensor_scalar(
                    out=s[:, sl].bitcast(mybir.dt.uint16),
                    in0=t[:, sl].bitcast(mybir.dt.uint16),
                    scalar1=0x7FFF, scalar2=None,
                    op0=OP.bitwise_and,
                )
                w = qpool.tile([128, T], f16)
                nc.vector.tensor_scalar(
                    out=w[:, sl], in0=s[:, sl], scalar1=G3, scalar2=G2,
                    op0=OP.mult, op1=OP.add,
                )
                w2 = qpool.tile([128, T], f16)
                nc.vector.tensor_tensor(
                    out=w2[:, sl], in0=w[:, sl], in1=s[:, sl], op=OP.mult
                )
                nc.vector.tensor_scalar(
                    out=w[:, sl], in0=w2[:, sl], scalar1=G1, scalar2=None,
                    op0=OP.add,
                )
                nc.vector.tensor_tensor(
                    out=w2[:, sl], in0=w[:, sl], in1=s[:, sl], op=OP.mult
                )
                nc.vector.tensor_scalar(
                    out=w[:, sl], in0=w2[:, sl], scalar1=G0, scalar2=None,
                    op0=OP.add,
                )
                nc.vector.tensor_tensor(
                    out=Y[:, sl], in0=w[:, sl], in1=t[:, sl], op=OP.mult
                )

            def exp_b(b):
                nc.scalar.activation(
                    out=E[:, b, :], in_=Pp[b][:], func=AF.Exp,
                )

            def transpose_all():
                # E free layout is (b, tc) -> one transpose gives the
                # (b,tc)-partition layout directly (PE transpose output
                # base partition must be 0/32/64, so per-b is impossible)
                nc.tensor.transpose(
                    out=attT[:],
                    in_=E[:].rearrange("p b tc -> p (b tc)"),
                    identity=ident[:],
                )

            def rowsum_all():
                with nc.allow_low_precision(reason="bf16 rowsums, f32 accum"):
                    nc.vector.tensor_reduce(
                        out=R128[:],
                        in_=attT[:],
                        axis=mybir.AxisListType.X,
                        op=OP.add,
                    )

            allc = list(range(NTC))
            Ys = {}

            def emit_act(b, hc, X=None, slices=((0, T),)):
                if X is None:
                    X = xpool.tile([128, T], f16, name="X")
                    nc.sync.dma_start(out=X[:], in_=encv[b, hc, :, :])
                Ys[(b, hc)] = act_tile(b, hc, X, slices)

            def pre_issue(b, hc):
                """gpsimd-ring (SWDGE) tile, issued well ahead of use."""
                X = xpool.tile([128, T], f16, name="X")
                nc.gpsimd.dma_start(out=X[:], in_=encv[b, hc, :, :])
                return X

            # Global emission order IS the dependency DAG (the framework
            # derives deps from data flow at emission time); each engine
            # executes its own subsequence in FIFO order.  This order is
            # arranged so no engine head-of-line-blocks a critical dep.

            # --- b0; the X02 scalar-ring issue rides right after the
            #     first ACTIVATE slice (transfer hits the fast window) ---
            Y00 = ypool.tile([128, T], f16, name="Y")
            nc.scalar.activation(
                out=Y00[:, 0:1024], in_=X00[:, 0:1024], func=AF.Tanh,
                bias=rec_sb[:, 0, 0:1],
            )
            nc.scalar.dma_start(out=Xdve[(0, 2)][:], in_=encv[0, 2, :, :])
            nc.scalar.activation(
                out=Y00[:, 1024:2048], in_=X00[:, 1024:2048], func=AF.Tanh,
                bias=rec_sb[:, 0, 0:1],
            )
            nc.scalar.activation(
                out=Y00[:, 2048:T], in_=X00[:, 2048:T], func=AF.Tanh,
                bias=rec_sb[:, 0, 0:1],
            )
            Ys[(0, 0)] = Y00
            score_cols(0, 0, Y00, allc, False)
            X01 = xpool.tile([128, T], f16, name="X")
            nc.sync.dma_start(out=X01[:, :2048], in_=encv[0, 1, :, :2048])
            nc.sync.dma_start(out=X01[:, 2048:], in_=encv[0, 1, :, 2048:])
            Ys[(0, 1)] = act_tile(0, 1, X01, [(0, 2048), (2048, T)])
            score_cols(0, 1, Ys[(0, 1)], allc, False)
            emit_act(0, 3)
            score_cols(0, 3, Ys[(0, 3)], allc, False)
            Y02 = ypool.tile([128, T], f16, name="Y")
            dve_tile(0, 2, Xdve[(0, 2)], Y02, 0, T)
            score_cols(0, 2, Y02, allc, True)            # closes Pp[0]

            # --- b1; SWDGE issues for later tiles spread through here ---
            emit_act(1, 0)
            nc.gpsimd.dma_start(out=Xdve[(1, 2)][:], in_=encv[1, 2, :, :])
            score_cols(1, 0, Ys[(1, 0)], allc, False)
            emit_act(1, 1)
            nc.gpsimd.dma_start(out=Xdve[(2, 2)][:], in_=encv[2, 2, :, :])
            score_cols(1, 1, Ys[(1, 1)], allc, False)
            emit_act(1, 3)
            X30 = pre_issue(3, 0)
            score_cols(1, 3, Ys[(1, 3)], allc, False)
            Y12 = ypool.tile([128, T], f16, name="Y")
            dve_tile(1, 2, Xdve[(1, 2)], Y12, 0, T)
            score_cols(1, 2, Y12, allc, True)            # closes Pp[1]

            # --- b2 (hc2 split: DVE does [0:2048], ACT does [2048:T]) ---
            emit_act(2, 0)
            X31 = pre_issue(3, 1)
            score_cols(2, 0, Ys[(2, 0)], allc, False)
            emit_act(2, 1)
            X32 = pre_issue(3, 2)
            score_cols(2, 1, Ys[(2, 1)], allc, False)
            emit_act(2, 3)
            nc.gpsimd.dma_start(out=mblk[:], in_=mblkd[:])
            score_cols(2, 3, Ys[(2, 3)], allc, False)
            Y22 = ypool.tile([128, T], f16, name="Y")
            nc.scalar.activation(
                out=Y22[:, 2048:T], in_=Xdve[(2, 2)][:, 2048:T], func=AF.Tanh,
                bias=rec_sb[:, 2, 2:3],
            )
            dve_tile(2, 2, Xdve[(2, 2)], Y22, 0, 2048)

            # --- b3 ---
            Ys[(3, 0)] = act_tile(3, 0, X30, [(0, T)])
            score_cols(3, 0, Ys[(3, 0)], allc, False)
            exp_b(0)
            Ys[(3, 1)] = act_tile(3, 1, X31, [(0, T)])
            score_cols(3, 1, Ys[(3, 1)], allc, False)
            exp_b(1)
            Ys[(3, 2)] = act_tile(3, 2, X32, [(0, T)])
            score_cols(3, 2, Ys[(3, 2)], allc, False)
            X33 = xpool.tile([128, T], f16, name="X")
            nc.sync.dma_start(out=X33[:], in_=encv[3, 3, :, :])
            Y33 = ypool.tile([128, T], f16, name="Y")
            nc.scalar.activation(
                out=Y33[:, 0:2048], in_=X33[:, 0:2048], func=AF.Tanh,
                bias=rec_sb[:, 3, 3:4],
            )
            score_cols(3, 3, Y33, list(range(16)), False)
            score_cols(2, 2, Y22, list(range(16, NTC)), True)  # ACT half
            nc.scalar.activation(
                out=Y33[:, 2048:T], in_=X33[:, 2048:T], func=AF.Tanh,
                bias=rec_sb[:, 3, 3:4],
            )
            score_cols(2, 2, Y22, list(range(16)), True)       # DVE half
            score_cols(3, 3, Y33, list(range(16, NTC)), True)  # closes Pp[3]
            exp_b(2)
            exp_b(3)
            transpose_all()
            rowsum_all()

            # DVE poly ops were emitted on the vector queue above; the
            # rowsums sit after them and before the final chain.

            # ---------------- final normalization ----------------
            denom = spool.tile([128, 1], f32, tag="denom")
            nc.tensor.matmul(
                denom[:], lhsT=mblk[:], rhs=R128[:], start=True, stop=True
            )
            recip = singles.tile([128, 1], f32)
            nc.vector.reciprocal(out=recip[:], in_=denom[:])
            att_out = singles.tile([128, 128], f32)
            nc.vector.tensor_scalar_mul(
                out=att_out[:], in0=attT[:], scalar1=recip[:]
            )
            # partition p = (b, tc) holds 128 contiguous t values for col b
            nc.sync.dma_start(
                out=out.rearrange("b (tc tp) -> (b tc) tp", tp=128),
                in_=att_out[:],
            )

    nc.compile()
    return nc


def _get_graph():
    global _GRAPH
    if _GRAPH is None:
        _GRAPH = _build_graph()
    return _GRAPH


def make_in_maps(enc, mask, rnn_state, W_rec, w_score):
    import ml_dtypes

    enc16 = np.asarray(enc, dtype=np.float16)
    # [T, B, H] -> [B, H, T]
    encT_full = np.ascontiguousarray(enc16.transpose(1, 2, 0))
    # [HC, 128, RC, 128]: wrecd[hc, p, rc, w] = W_rec[hc*128+w, rc*128+p]
    wrecd = np.ascontiguousarray(
        W_rec.T.astype(np.float16)
        .reshape(RC, 128, HC, 128)
        .transpose(2, 1, 0, 3)
    )
    wcol = np.ascontiguousarray(
        np.asarray(w_score, dtype=np.float16).reshape(HC, 128).T
    )
    # block-diag ones: mblk[p, q] = (p//32 == q//32)
    mblk = (np.arange(128)[:, None] // 32 == np.arange(128)[None, :] // 32)
    mblk = np.ascontiguousarray(mblk.astype(ml_dtypes.bfloat16))
    in_maps = []
    for c in range(NCORES):
        sl = slice(c * BL, (c + 1) * BL)
        # rnnd[p, rc, b] = rnn_state[c*BL+b, rc*128+p]
        rnnd = np.ascontiguousarray(
            rnn_state[sl].astype(np.float16).T.reshape(RC, 128, BL)
            .transpose(1, 0, 2)
        )
        # maskd[p, b, tc] = mask[tc*128+p, c*BL+b]
        maskc = np.ascontiguousarray(
            np.asarray(mask[:, sl], dtype=np.float32).reshape(NTC, 128, BL)
            .transpose(1, 2, 0)
        )
        in_maps.append(
            {
                "encT": np.ascontiguousarray(encT_full[sl]),
                "maskd": maskc,
                "rnnd": rnnd,
                "wrecd": wrecd,
                "wcold": wcol,
                "mblk": mblk,
            }
        )
    return in_maps


def kernel(
    encoded_contribution,
    mask,
    rnn_state,
    prev_att_weights,
    W_rec,
    w_score,
    b_score,
):
    from concourse.bass_utils import run_bass_kernel_spmd

    nc = _get_graph()
    in_maps = make_in_maps(
        np.asarray(encoded_contribution),
        np.asarray(mask),
        np.asarray(rnn_state),
        np.asarray(W_rec),
        np.asarray(w_score),
    )
    res = run_bass_kernel_spmd(nc, in_maps, list(range(NCORES)))
    outs = [np.asarray(res.results[c]["out"]) for c in range(NCORES)]
    return np.concatenate([o.T for o in outs], axis=1).astype(np.float32)
